# revision 36
# baseline (speedup 1.0000x reference)
"""DeltaMPredictor Trainium2 kernel (8 NeuronCores, data-parallel over batch).

Pipeline per token (b, c):
    reg = thumb @ proj_w.T + proj_b            [2048] -> [512]
    y   = (reg - mean) * rstd                  per-camera LayerNorm (gamma/beta
                                               folded into the SwiGLU weights)
    gate = y @ (w_gate*gamma).T + w_gate@beta
    val  = y @ (w_val *gamma).T + w_val @beta
    h   = silu(gate) * val
    A   = reshape(h @ w_out.T, 6, 6); A -= A.T; clip frob to 3
    dM  = expm(A)  (Horner degree-4 Taylor + 3 squarings on DVE/GPSIMD)

Sharding: batch B=16384 split 8 ways (2048 rows/core); all weights replicated.
Per core the loop is camera-major (4 cameras x 4 tiles of 512 tokens).

All matmul operands are bf16 (inputs quantized on host; fp32 PSUM accum);
the expm state is fp16 in an (x, y, g)-inner layout so every elementwise op
has contiguous inner runs and 2-byte DVE throughput.

Schedule (per iteration t): mm1(t) g0 + LN stats, then tile t-1's PE
transposes + DVE yT copies (deps one tile old -> never stall), mm1(t) g1-3
with inline LN, then mm2/mm3/silu/h of t-1, then the expm of tile t-2 is
emitted as pre-resolved op chunks that fill engine-queue slack without
head-of-line-blocking the next tile's LN chain.  The final tile's expm runs
a latency-optimized all-DVE chain (Newton rsqrt, no cross-engine hops).
"""

import os
import sys

sys.path.insert(0, "/opt/trn_rl_repo")

from contextlib import ExitStack

import ml_dtypes
import numpy as np

import concourse.bacc as bacc
import concourse.bass as bass
import concourse.tile as tile
from concourse import mybir
from concourse.bass_utils import run_bass_kernel_spmd
from concourse.masks import make_identity

B, C, D_BB, D = 16384, 4, 2048, 512
N_CORES = 8
BLOC = B // N_CORES          # 2048 batch rows per core
TOK = 512                    # tokens per tile
NT = BLOC // TOK             # 4 tiles per camera
G = TOK // 128               # 4 token chunks of 128 per tile
KE = D_BB // 128             # 16 contraction chunks for mm1
KD = D // 128                # 4 contraction chunks for mm2/mm3
MAX_NORM = 3.0
LN_EPS = 1e-5
EXP_S = 3                    # squarings in expm

F32 = mybir.dt.float32
F32R = mybir.dt.float32r
B16 = mybir.dt.bfloat16
I32 = mybir.dt.int32
AL = mybir.AluOpType
AF = mybir.ActivationFunctionType
AX = mybir.AxisListType

_BUILD_CACHE = {}
last_results = None          # test harness introspection
last_in_maps = None


def _emit_rsqrt(nc, pool, out, x, n, tag, iters=2, eng=None):
    """out = 1/sqrt(x) elementwise for [128, n] fp32 SBUF tiles.

    Magic-constant seed + `iters` Newton steps (rel err ~4e-6 at 2 iters).
    Safe for x == 0 (result is finite-huge, no NaN).
    """
    if eng is None:
        eng = nc.vector
    magic = pool.tile([128, 1], I32, tag=f"{tag}_magic")
    nc.vector.memset(magic, 0x5F3759DF)
    sh = pool.tile([128, n], I32, tag=f"{tag}_sh")
    eng.tensor_scalar(
        out=sh, in0=x.bitcast(I32), scalar1=1, scalar2=None,
        op0=AL.logical_shift_right,
    )
    eng.tensor_tensor(
        out=out.bitcast(I32),
        in0=magic[:, 0:1].broadcast_to((128, n)),
        in1=sh,
        op=AL.subtract,
    )
    tmp = pool.tile([128, n], F32, tag=f"{tag}_tmp")
    for _ in range(iters):
        eng.tensor_tensor(out=tmp, in0=x, in1=out, op=AL.mult)
        eng.tensor_tensor(out=tmp, in0=tmp, in1=out, op=AL.mult)
        eng.tensor_scalar(
            out=tmp, in0=tmp, scalar1=-0.5, scalar2=1.5, op0=AL.mult, op1=AL.add
        )
        eng.tensor_tensor(out=out, in0=out, in1=tmp, op=AL.mult)


def _emit_expm(nc, pool, A0, neghalf_c, out_slice, latency_mode=False):
    """A0 [128, G, 36] fp32 (g-major): skew, frob-clip, expm into out_slice
    [128, G*36] (g-major).

    Returns a list of zero-arg closures ("chunks") that emit the actual ops;
    the caller drains them interleaved with the next tile's mm1/LN emission so
    latency-critical LN ops never queue behind a full tile's expm burst.

    Internally the state lives in (x, y, g) layout (g innermost): a 6x6
    matmul product C = A @ B uses left operand L[p,i,l,g] and right operand
    R[p,j,l,g] = B^T, so the broadcast multiply is ONE op over all 4 g-groups
    with contiguous inner runs, and the l-reduction is 3 pairwise adds.
    Skew-symmetry makes the first right operand free: As^T = -As.
    """
    X16 = mybir.dt.float16   # expm state dtype: 2-byte for 2x DVE modes
    # latency_mode (final tile): keep the whole chain on the DVE so the tail
    # has no cross-engine queue hops; rsqrt switches to a Newton iteration.
    ge = nc.vector if latency_mode else nc.gpsimd

    def fl(t):
        return t[:, :, :, :].rearrange("p x y g -> p (x y g)")

    S_A = pool.tile([128, 6, 6, G], X16, tag="xS")        # (i, l, g)
    SQ = pool.tile([128, 6 * 6 * G], X16, tag="xSQ")
    Rr = pool.tile([128, 6 * G], F32, tag="xR")
    ss = pool.tile([128, G], F32, tag="xss")
    rsq = pool.tile([128, G], F32, tag="xrsq")
    scs = pool.tile([128, G], F32, tag="xscs")
    nscs = pool.tile([128, G], F32, tag="xnscs")
    As_A = pool.tile([128, 6, 6, G], X16, tag="xAsA")     # As, (i, l, g)
    As_T = pool.tile([128, 6, 6, G], X16, tag="xAsT")     # As^T = -As

    a0v = A0[:, :, :].rearrange("p g (i j) -> p g i j", i=6)

    def chunk_head():
        # skew into (i, l, g): S[p,i,l,g] = A0[p,g,i,l] - A0[p,g,l,i]
        ge.tensor_tensor(
            out=S_A[:, :, :, :],
            in0=a0v.transpose([0, 2, 3, 1]),
            in1=a0v.transpose([0, 3, 2, 1]),
            op=AL.subtract,
        )
        ge.tensor_tensor(
            out=SQ[:, :], in0=fl(S_A), in1=fl(S_A), op=AL.mult
        )
        # ss[g] = sum_{i,l} S^2: two gather reduces (DVE owns free-dim reduce)
        nc.vector.tensor_reduce(
            out=Rr[:, :],
            in_=SQ[:, :].rearrange("p (i lg) -> p lg i", i=6),
            axis=AX.X, op=AL.add,
        )
        nc.vector.tensor_reduce(
            out=ss[:, :],
            in_=Rr[:, :].rearrange("p (l g) -> p g l", l=6),
            axis=AX.X, op=AL.add,
        )
        if latency_mode:
            _emit_rsqrt(nc, pool, rsq[:, :], ss[:, :], G, "lmr", eng=nc.vector)
        else:
            nc.gpsimd.tensor_tensor(
                out=rsq[:, :], in0=ss[:, :],
                in1=neghalf_c[:, 0:1].broadcast_to((128, G)), op=AL.pow,
            )
        # scs = min(MAX_NORM * rsqrt(ss), 1) / 2^EXP_S
        nc.vector.tensor_scalar(
            out=scs[:, :], in0=rsq[:, :],
            scalar1=MAX_NORM / (1 << EXP_S), scalar2=1.0 / (1 << EXP_S),
            op0=AL.mult, op1=AL.min,
        )
        nc.vector.tensor_scalar(
            out=nscs[:, :], in0=scs[:, :], scalar1=-1.0, scalar2=None,
            op0=AL.mult,
        )
        sbc = scs[:, :].unsqueeze(1).broadcast_to((128, 36, G))
        nbc = nscs[:, :].unsqueeze(1).broadcast_to((128, 36, G))
        sv = S_A[:, :, :, :].rearrange("p i l g -> p (i l) g")
        ge.tensor_tensor(
            out=As_A[:, :, :, :].rearrange("p i l g -> p (i l) g"),
            in0=sv, in1=sbc, op=AL.mult,
        )
        ge.tensor_tensor(
            out=As_T[:, :, :, :].rearrange("p i l g -> p (i l) g"),
            in0=sv, in1=nbc, op=AL.mult,
        )

    def diag_add_one(t):
        dv = t[:, :, :, :].rearrange("p x y g -> p (x y) g")[:, 0 : 36 : 7, :]
        nc.vector.tensor_scalar(
            out=dv, in0=dv, scalar1=1.0, scalar2=None, op0=AL.add
        )

    TMP = pool.tile([128, 6, 6, 6, G], X16, tag="xTMP", bufs=2)  # (i,j,l,g)
    T2 = pool.tile([128, 36, 3 * G], X16, tag="xT2", bufs=2)     # (ij, l<3, g)

    # latency_mode: split each bprod by output row i into two parallel
    # engine chains (DVE i0:4, GPSIMD i4:6) to shrink the tail's latency.
    ISL = [(0, 6, None)]

    def bprod(out_ap, left, right, out_ap2=None):
        """out = left @ right per (token, g); out_ap (and the optional
        second output, e.g. the transposed view for the next squaring's
        right operand) is a 4-dim (p,i,j,g) view with any strides."""
        T3 = pool.tile([128, 36, G], X16, tag="xT3")
        t2v = T2[:, :, :].rearrange("p (i j) (l g) -> p i j l g", i=6, l=3)
        for i0, i1, e in ISL:
            ni = i1 - i0
            ve = nc.gpsimd if e == "g" else nc.vector
            ae = ve if e is not None else ge
            lv = (
                left[:, i0:i1, :, :].rearrange("p i l g -> p i (l g)")
                .unsqueeze(2).broadcast_to((128, ni, 6, 6 * G))
            )
            rv = (
                right[:, :, :, :].rearrange("p j l g -> p j (l g)")
                .unsqueeze(1).broadcast_to((128, ni, 6, 6 * G))
            )
            tv = TMP[:, i0:i1, :, :, :].rearrange("p i j l g -> p i j (l g)")
            ve.tensor_tensor(out=tv, in0=lv, in1=rv, op=AL.mult)
            tq = TMP[:, i0:i1, :, :, :].rearrange("p i j l g -> p (i j) l g")
            t2s = t2v[:, i0:i1, :, :, :].rearrange("p i j l g -> p (i j) l g")
            ae.tensor_tensor(
                out=t2s, in0=tq[:, :, 0:3, :], in1=tq[:, :, 3:6, :], op=AL.add
            )
            t3s = T3[:, i0 * 6 : i1 * 6, :]
            ve.tensor_tensor(
                out=t3s, in0=t2s[:, :, 0, :], in1=t2s[:, :, 1, :], op=AL.add
            )
            t2last = t2s[:, :, 2, :].rearrange("p (i j) g -> p i j g", i=ni)
            t3v_ = t3s.rearrange("p (i j) g -> p i j g", i=ni)
            ve.tensor_tensor(
                out=out_ap[:, i0:i1, :, :], in0=t3v_, in1=t2last, op=AL.add
            )
            if out_ap2 is not None:
                ve.tensor_tensor(
                    out=out_ap2[:, i0:i1, :, :], in0=t3v_, in1=t2last,
                    op=AL.add,
                )

    def rm(t):   # row-major (i, j, g) view of a state tile
        return t[:, :, :, :]

    def cm(t):   # column-major view: write C^T by swapping i/j dims
        return t[:, :, :, :].transpose([0, 2, 1, 3])

    chunks = [chunk_head]

    # Horner: M = I + As/4; for k=3..2: M = I + (As@M)/k; E = I + As@M.
    # M kept transposed (right-operand layout); As_T = M0 seed via skewness.
    M0 = pool.tile([128, 6, 6, G], X16, tag="xM0")

    def chunk_m0():
        nc.vector.tensor_scalar(
            out=fl(M0), in0=fl(As_T), scalar1=1.0 / 4.0, scalar2=None,
            op0=AL.mult,
        )
        diag_add_one(M0)

    chunks.append(chunk_m0)

    prevM = [M0]
    for k in (3, 2):
        Mk = pool.tile([128, 6, 6, G], X16, tag=f"xM{k}")

        def chunk_h(Mk=Mk, Mprev=prevM[0], k=k):
            bprod(cm(Mk), As_A, Mprev)       # Mk := (As@Mprev)^T
            nc.vector.tensor_scalar(
                out=fl(Mk), in0=fl(Mk), scalar1=1.0 / k, scalar2=None,
                op0=AL.mult,
            )
            diag_add_one(Mk)

        chunks.append(chunk_h)
        prevM[0] = Mk

    E_rm = pool.tile([128, 6, 6, G], X16, tag="xE0r")
    E_cm = pool.tile([128, 6, 6, G], X16, tag="xE0c")

    def chunk_e0(Mprev=prevM[0]):
        bprod(rm(E_rm), As_A, Mprev, out_ap2=cm(E_cm))
        diag_add_one(E_rm)
        diag_add_one(E_cm)

    chunks.append(chunk_e0)

    prevE = [(E_rm, E_cm)]
    for s in range(EXP_S):
        if s < EXP_S - 1:
            Er = pool.tile([128, 6, 6, G], X16, tag=f"xE{s+1}r")
            Ec = pool.tile([128, 6, 6, G], X16, tag=f"xE{s+1}c")

            def chunk_sq(Er=Er, Ec=Ec, prev=prevE[0]):
                bprod(rm(Er), prev[0], prev[1], out_ap2=cm(Ec))

            chunks.append(chunk_sq)
            prevE[0] = (Er, Ec)
        else:
            out_v = out_slice.rearrange("p (g i j) -> p i j g", i=6, j=6)

            def chunk_last(prev=prevE[0], out_v=out_v):
                bprod(out_v, prev[0], prev[1])

            chunks.append(chunk_last)
    return chunks


def _build(emit_pb, emit_gb):
    nc = bacc.Bacc("TRN2", target_bir_lowering=False, debug=False)

    th = nc.dram_tensor("th", [C, NT, 128, KE * TOK], B16, kind="ExternalInput")
    pwT = nc.dram_tensor("pwT", [128, KE * D], B16, kind="ExternalInput")
    wgT = nc.dram_tensor("wgT", [C, 128, KD * D], B16, kind="ExternalInput")
    wvT = nc.dram_tensor("wvT", [C, 128, KD * D], B16, kind="ExternalInput")
    woT = nc.dram_tensor("woT", [C, 128, KD * 36], B16, kind="ExternalInput")
    pb = bg = bv = None
    if emit_pb:
        pb = nc.dram_tensor("pb", [1, D], F32R, kind="ExternalInput")
    if emit_gb:
        bg = nc.dram_tensor("bg", [C, D], F32R, kind="ExternalInput")
        bv = nc.dram_tensor("bv", [C, D], F32R, kind="ExternalInput")
    out = nc.dram_tensor("out", [128, C * NT * G * 36], F32, kind="ExternalOutput")

    with tile.TileContext(nc) as tc, ExitStack() as ctx:
        singles = ctx.enter_context(tc.tile_pool(name="singles", bufs=1))
        cam = ctx.enter_context(tc.tile_pool(name="cam", bufs=2))
        tkp = ctx.enter_context(tc.tile_pool(name="tkp", bufs=2))
        work = ctx.enter_context(tc.tile_pool(name="work", bufs=2))
        xw = ctx.enter_context(tc.tile_pool(name="xw", bufs=3))
        # PSUM budget (8 banks): transposes+a0 share 1, mm1-reg 4, gate/val 3.
        tr_ps = ctx.enter_context(tc.tile_pool(name="tr_ps", bufs=2, space="PSUM"))
        reg_psp = ctx.enter_context(tc.tile_pool(name="reg_ps", bufs=3, space="PSUM"))
        mm_ps = ctx.enter_context(tc.tile_pool(name="mm_ps", bufs=3, space="PSUM"))

        # memset cannot emit float32r directly; stage in f32 and cast-copy.
        identF = singles.tile([128, 128], F32)
        make_identity(nc, identF)
        identB = singles.tile([128, 128], B16)
        nc.vector.tensor_copy(identB, identF)
        pw_s = singles.tile([128, KE, D], B16)
        pwsrc = pwT.ap().rearrange("p (k d) -> p k d", k=KE)

        for q in range(4):
            nc.gpsimd.dma_start(
                pw_s[:, q * 4 : (q + 1) * 4, :],
                pwsrc[:, q * 4 : (q + 1) * 4, :],
            )
        onesF = singles.tile([1, TOK], F32)
        nc.vector.memset(onesF, 1.0)
        ones128 = singles.tile([1, 128], F32R)
        nc.vector.tensor_copy(ones128, onesF[:, :128])
        ones512 = singles.tile([1, TOK], F32R)
        nc.vector.tensor_copy(ones512, onesF)
        pb_s = None
        if emit_pb:
            pb_s = singles.tile([1, D], F32R)
            nc.gpsimd.dma_start(pb_s, pb.ap())
        # all 16 tiles' results accumulate here; one DMA at the end keeps the
        # SP queue free of expm-dependent waits mid-stream.
        eacc = singles.tile([128, C * NT, G * 36], F32)
        # gpsimd rsqrt via vpowf: pow(x, -0.5)
        neghalf = singles.tile([128, 1], F32)
        nc.vector.memset(neghalf, -0.5)
        negone = singles.tile([128, 1], F32)
        nc.vector.memset(negone, -1.0)
        epsB = singles.tile([128, 1], F32)
        nc.vector.memset(epsB, LN_EPS)
        invD = singles.tile([128, 1], F32)
        nc.vector.memset(invD, 1.0 / D)

        cam_state = {}

        def stage_a(step, tr_hook=None):
            """thumbnail DMA + mm1 + LayerNorm for tile `step` -> y."""
            c, ti = divmod(step, NT)
            if ti == 0:
                wg_s = cam.tile([128, KD, D], B16, tag="wg")
                nc.gpsimd.dma_start(
                    wg_s, wgT.ap()[c].rearrange("p (k f) -> p k f", k=KD)
                )
                wv_s = cam.tile([128, KD, D], B16, tag="wv")
                nc.gpsimd.dma_start(
                    wv_s, wvT.ap()[c].rearrange("p (k f) -> p k f", k=KD)
                )
                wo_s = cam.tile([128, KD, 36], B16, tag="wo")
                nc.gpsimd.dma_start(
                    wo_s, woT.ap()[c].rearrange("p (k o) -> p k o", k=KD)
                )
                bg_s = bv_s = None
                if emit_gb:
                    bg_s = cam.tile([1, D], F32R, tag="bg")
                    nc.gpsimd.dma_start(bg_s, bg.ap()[c : c + 1, :])
                    bv_s = cam.tile([1, D], F32R, tag="bv")
                    nc.gpsimd.dma_start(bv_s, bv.ap()[c : c + 1, :])
                cam_state[c] = (wg_s, wv_s, wo_s, bg_s, bv_s)

            # thumbnails arrive host-blocked [C, NT, 128, KE*TOK]: one DMA
            # with one contiguous run per partition, no on-chip transpose.
            thsrc = th.ap()[c, ti].rearrange("p (k b) -> p k b", k=KE)
            q = KE // 4
            ths = []
            for j in range(4):
                t_ = tkp.tile([128, q, TOK], B16, tag=f"tk{j}")
                # two queues: halves the per-queue DMA descriptor serialization
                eng = nc.sync if j % 2 == 0 else nc.scalar
                eng.dma_start(t_, thsrc[:, j * q : (j + 1) * q, :])
                ths.append(t_)

            # ---- mm1 (m-outer, one PSUM bank per token chunk) + LayerNorm.
            # Per chunk: 16 accumulating matmuls, bn stats on DVE, rsqrt
            # chain on GPSIMD, then y = rstd*reg - mu*rstd on ACT (frees
            # the bank early so chunks pipeline through 3 reg banks).
            y = work.tile([128, G, D], B16, tag="y")
            rstd = work.tile([128, G], F32, tag="rstd")
            for g in range(G):
                reg_ps = reg_psp.tile([128, D], F32, tag="reg")
                for k in range(KE):
                    tsrc = ths[k // q]
                    nc.tensor.matmul(
                        reg_ps,
                        tsrc[:, k % q, g * 128 : (g + 1) * 128],
                        pw_s[:, k, :],
                        start=(k == 0),
                        stop=(k == KE - 1 and not emit_pb),
                    )
                if emit_pb:
                    nc.tensor.matmul(reg_ps, ones128, pb_s, start=False, stop=True)
                st = work.tile([128, 6], F32, tag="bst")
                nc.vector.bn_stats(out=st[:, :], in_=reg_ps)
                mv = work.tile([128, 2], F32, tag="mv")
                nc.vector.bn_aggr(out=mv[:, :], in_=st[:, :])
                negmu = work.tile([128, 1], F32, tag="negmu")
                nc.vector.tensor_scalar(
                    out=negmu, in0=mv[:, 0:1], scalar1=-1.0,
                    scalar2=None, op0=AL.mult,
                )
                # rstd = pow(var+eps, -0.5) via gpsimd vpowf, then
                # y = rstd*reg - mu*rstd in one ACT op (single rounding,
                # frees this chunk's PSUM bank right away)
                vpe = work.tile([128, 1], F32, tag="vpe")
                nc.gpsimd.tensor_tensor(out=vpe, in0=mv[:, 1:2], in1=epsB, op=AL.add)
                nc.gpsimd.tensor_tensor(
                    out=rstd[:, g : g + 1], in0=vpe, in1=neghalf, op=AL.pow
                )
                nmr = work.tile([128, 1], F32, tag="nmr")
                nc.gpsimd.tensor_tensor(
                    out=nmr, in0=negmu, in1=rstd[:, g : g + 1], op=AL.mult
                )
                if g == 0 and tr_hook is not None:
                    # previous tile's transposes + yT copies: their deps are a
                    # tile old, so they fill the ACT queue ahead of y(t) and
                    # free the tr PSUM banks early
                    tr_hook()
                nc.scalar.activation(
                    y[:, g, :], reg_ps, AF.Identity,
                    bias=nmr, scale=rstd[:, g : g + 1],
                )
            return {"c": c, "ti": ti, "y": y}

        def stage_b1(st):
            """previous tile's yT transposes + copies (PE + ACT)."""
            y = st["y"]
            yTs = []
            for kd in range(KD):
                tr = tr_ps.tile([128, TOK], B16, tag="tr")
                for g in range(G):
                    nc.tensor.transpose(
                        tr[:, g * 128 : (g + 1) * 128],
                        y[:, g, kd * 128 : (kd + 1) * 128],
                        identB,
                    )
                yT = work.tile([128, TOK], B16, tag=f"yT{kd}", bufs=1)
                # kd3's copy gates the next tile's first transpose via PSUM
                # bank reuse: keep it on the prompt ACT lane, rest on DVE
                if kd == KD - 1:
                    nc.scalar.copy(yT[:, :], tr)
                else:
                    nc.vector.tensor_copy(yT[:, :], tr)
                yTs.append(yT)
            st["yTs"] = yTs

        def stage_b2(st, latency_mode=False):
            """SwiGLU + out-proj + expm for a completed tile."""
            c, ti = st["c"], st["ti"]
            yTs = st["yTs"]
            wg_s, wv_s, wo_s, bg_s, bv_s = cam_state[c]

            # ---- mm2 gate/val + silu + h
            hs = []
            for mf in range(KD):
                g_ps = mm_ps.tile([128, TOK], F32, tag="mm")
                for kd in range(KD):
                    nc.tensor.matmul(
                        g_ps,
                        wg_s[:, kd, mf * 128 : (mf + 1) * 128],
                        yTs[kd][:, :],
                        start=(kd == 0),
                        stop=(kd == KD - 1 and not emit_gb),
                    )
                if emit_gb:
                    nc.tensor.matmul(
                        g_ps, bg_s[:, mf * 128 : (mf + 1) * 128], ones512,
                        start=False, stop=True,
                    )
                v_ps = mm_ps.tile([128, TOK], F32, tag="mm")
                for kd in range(KD):
                    nc.tensor.matmul(
                        v_ps,
                        wv_s[:, kd, mf * 128 : (mf + 1) * 128],
                        yTs[kd][:, :],
                        start=(kd == 0),
                        stop=(kd == KD - 1 and not emit_gb),
                    )
                if emit_gb:
                    nc.tensor.matmul(
                        v_ps, bv_s[:, mf * 128 : (mf + 1) * 128], ones512,
                        start=False, stop=True,
                    )
                sg = work.tile([128, TOK], F32, tag="sg", bufs=2)
                nc.scalar.activation(sg, g_ps, AF.Silu)
                h = work.tile([128, TOK], B16, tag=f"h{mf}", bufs=1)
                nc.vector.tensor_tensor(out=h[:, :], in0=sg, in1=v_ps, op=AL.mult)
                hs.append(h)

            # ---- mm3: A0T[36, t] = woT.T @ h (4 wide fp32r matmuls),
            # then PE-transpose back to [t, 36] for the expm stage.
            a0_ps = tr_ps.tile([128, G, 36], B16, tag="tr")
            a0T_ps = mm_ps.tile([36, TOK], F32, tag="mm", name="a0T")
            for kf in range(KD):
                nc.tensor.matmul(
                    a0T_ps,
                    wo_s[:, kf, :],
                    hs[kf][:, :],
                    start=(kf == 0),
                    stop=(kf == KD - 1),
                )
            a0T_s = work.tile([36, TOK], B16, tag="a0T")
            nc.scalar.copy(a0T_s, a0T_ps)
            for g in range(G):
                nc.tensor.transpose(
                    a0_ps[:, g, :],
                    a0T_s[:, g * 128 : (g + 1) * 128],
                    identB[:36, :36],
                )
            A0 = xw.tile([128, G, 36], mybir.dt.float16, tag="A0")
            if latency_mode:
                nc.vector.tensor_copy(A0[:, :, :], a0_ps[:, :, :])
            else:
                nc.scalar.copy(A0[:, :, :], a0_ps[:, :, :])
            return _emit_expm(
                nc, xw, A0, neghalf, out_slice=eacc[:, c * NT + ti, :],
                latency_mode=latency_mode,
            )

        # 1-stage software pipeline: tile t's post-mm1 work (PE transposes,
        # mm2/mm3) is emitted after tile t+1's mm1+LN, so the PE never waits
        # on the LN chain.  The expm of tile t is emitted as chunks
        # interleaved into tile t+2's mm1/LN g-loop so latency-critical LN
        # ops never queue behind a full expm burst on any in-order engine.
        pending = []          # (tile_idx, closure) FIFO
        cur_step = [0]

        def drain(k, force=False):
            for _ in range(k):
                if not pending:
                    return
                if not force and pending[0][0] > cur_step[0] - 2:
                    return
                pending.pop(0)[1]()

        prev = None
        for step in range(C * NT):
            cur_step[0] = step
            if step >= 6 and (step - 6) % NT == 0:
                cc = (step - 6) // NT
                nc.sync.dma_start(
                    out.ap()[:, cc * NT * G * 36 : (cc + 1) * NT * G * 36],
                    eacc[:, cc * NT : (cc + 1) * NT, :].rearrange(
                        "p t a -> p (t a)"
                    ),
                )
            hook = (lambda p=prev: stage_b1(p)) if prev is not None else None
            cur = stage_a(step, tr_hook=hook)
            if prev is not None:
                pending.extend(
                    (step - 1, ch) for ch in stage_b2(prev)
                )
            drain(len(pending))
            prev = cur
        drain(len(pending), force=True)   # tile 14's expm overlaps tile 15's PE
        stage_b1(prev)
        pending.extend(
            (C * NT - 1, ch) for ch in stage_b2(prev, latency_mode=True)
        )
        drain(len(pending), force=True)

        nc.sync.dma_start(
            out.ap()[:, 3 * NT * G * 36 :],
            eacc[:, 3 * NT :, :].rearrange("p t a -> p (t a)"),
        )

    nc.compile()
    return nc


def kernel(**inputs):
    global last_results, last_in_maps
    thumb = np.asarray(inputs["thumbnails"], dtype=np.float32)
    # [B, C, E] -> per-core [C, NT, 128, KE, TOK] so each tile lands with one
    # DMA whose per-partition data is contiguous (128 descriptors, not 2048).
    thB = thumb.reshape(N_CORES, NT, TOK, C, KE, 128)
    thB = np.ascontiguousarray(thB.transpose(0, 3, 1, 5, 4, 2))
    proj_w = np.asarray(inputs["proj_w"], dtype=np.float32)
    proj_b = np.asarray(inputs["proj_b"], dtype=np.float32)
    gamma = np.asarray(inputs["gamma"], dtype=np.float32)
    beta = np.asarray(inputs["beta"], dtype=np.float32)
    w_gate = np.asarray(inputs["w_gate"], dtype=np.float32)
    w_val = np.asarray(inputs["w_val"], dtype=np.float32)
    w_out = np.asarray(inputs["w_out"], dtype=np.float32)

    # host-side weight prep: fold gamma into the SwiGLU weights, beta into
    # rank-1 biases, pre-transpose everything for the PE's lhsT convention.
    def blockT(w):
        # [out, in] -> in-major [128, k, out]: lhsT chunks contiguous/partition
        o, i = w.shape
        return np.ascontiguousarray(
            w.T.reshape(i // 128, 128, o).transpose(1, 0, 2)
        ).reshape(128, i // 128 * o)

    BF = ml_dtypes.bfloat16
    pwT = blockT(proj_w).astype(BF)                             # [128, KE*D]
    wgT = np.stack(
        [blockT(w_gate[c] * gamma[c][None, :]) for c in range(C)]
    ).astype(BF)
    wvT = np.stack(
        [blockT(w_val[c] * gamma[c][None, :]) for c in range(C)]
    ).astype(BF)
    woT = np.stack([blockT(w_out[c]) for c in range(C)]).astype(BF)
    bg = np.einsum("cfd,cd->cf", w_gate, beta).astype(np.float32)
    bv = np.einsum("cfd,cd->cf", w_val, beta).astype(np.float32)

    emit_pb = bool(np.any(proj_b))
    emit_gb = bool(np.any(bg) or np.any(bv))

    key = (emit_pb, emit_gb)
    if key not in _BUILD_CACHE:
        _BUILD_CACHE[key] = _build(emit_pb, emit_gb)
    nc = _BUILD_CACHE[key]

    shared = {"pwT": pwT, "wgT": wgT, "wvT": wvT, "woT": woT}
    if emit_pb:
        shared["pb"] = proj_b.reshape(1, D)
    if emit_gb:
        shared["bg"] = bg
        shared["bv"] = bv
    in_maps = []
    for i in range(N_CORES):
        m = dict(shared)
        m["th"] = thB[i].reshape(C, NT, 128, KE * TOK).astype(ml_dtypes.bfloat16)
        in_maps.append(m)

    last_in_maps = in_maps
    trace = bool(int(os.environ.get("KERNEL_TRACE", "0")))
    try:
        last_results = run_bass_kernel_spmd(
            nc, in_maps, core_ids=list(range(N_CORES)), trace=trace
        )
    except ModuleNotFoundError:
        # tracing requested (e.g. BASS_TRACE in env) but the axon NTFF hook
        # module is absent in this image -- rerun without tracing.
        os.environ["BASS_NEVER_TRACE"] = "1"
        last_results = run_bass_kernel_spmd(
            nc, in_maps, core_ids=list(range(N_CORES)), trace=False
        )
    parts = []
    for r in last_results.results:
        o = r["out"].reshape(128, C, NT, G, 36)
        parts.append(o.transpose(1, 2, 3, 0, 4).reshape(C, BLOC, 36))
    full = np.concatenate(parts, axis=1)                        # [C, B, 36]
    return full.reshape(C, B, 6, 6)



# revision 37
# speedup vs baseline: 1.1025x; 1.1025x over previous
"""DeltaMPredictor Trainium2 kernel (8 NeuronCores, data-parallel over batch).

Pipeline per token (b, c):
    reg = thumb @ proj_w.T + proj_b            [2048] -> [512]
    y   = (reg - mean) * rstd                  per-camera LayerNorm (gamma/beta
                                               folded into the SwiGLU weights)
    gate = y @ (w_gate*gamma).T + w_gate@beta
    val  = y @ (w_val *gamma).T + w_val @beta
    h   = silu(gate) * val
    A   = reshape(h @ w_out.T, 6, 6); A -= A.T; clip frob to 3
    dM  = expm(A)  (Horner degree-4 Taylor + 3 squarings on DVE/GPSIMD)

Sharding: batch B=16384 split 8 ways (2048 rows/core); all weights replicated.
Per core the loop is camera-major (4 cameras x 4 tiles of 512 tokens).

All matmul operands are bf16 (inputs quantized on host; fp32 PSUM accum);
the expm state is fp16 in an (x, y, g)-inner layout so every elementwise op
has contiguous inner runs and 2-byte DVE throughput.

Schedule (per iteration t): mm1(t) g0 + LN stats, then tile t-1's PE
transposes + DVE yT copies (deps one tile old -> never stall), mm1(t) g1-3
with inline LN, then mm2/mm3/silu/h of t-1, then the expm of tile t-2 is
emitted as pre-resolved op chunks that fill engine-queue slack without
head-of-line-blocking the next tile's LN chain.  The final tile's expm runs
a latency-optimized all-DVE chain (Newton rsqrt, no cross-engine hops).
"""

import os
import sys

sys.path.insert(0, "/opt/trn_rl_repo")

from contextlib import ExitStack

import ml_dtypes
import numpy as np

import concourse.bacc as bacc
import concourse.bass as bass
import concourse.tile as tile
from concourse import mybir
from concourse.bass_utils import run_bass_kernel_spmd
from concourse.masks import make_identity

B, C, D_BB, D = 16384, 4, 2048, 512
N_CORES = 8
BLOC = B // N_CORES          # 2048 batch rows per core
TOK = 512                    # tokens per tile
NT = BLOC // TOK             # 4 tiles per camera
G = TOK // 128               # 4 token chunks of 128 per tile
KE = D_BB // 128             # 16 contraction chunks for mm1
KD = D // 128                # 4 contraction chunks for mm2/mm3
MAX_NORM = 3.0
LN_EPS = 1e-5
EXP_S = 3                    # squarings in expm

F32 = mybir.dt.float32
F32R = mybir.dt.float32r
B16 = mybir.dt.bfloat16
I32 = mybir.dt.int32
AL = mybir.AluOpType
AF = mybir.ActivationFunctionType
AX = mybir.AxisListType

_BUILD_CACHE = {}
last_results = None          # test harness introspection
last_in_maps = None


def _emit_rsqrt(nc, pool, out, x, n, tag, iters=2, eng=None):
    """out = 1/sqrt(x) elementwise for [128, n] fp32 SBUF tiles.

    Magic-constant seed + `iters` Newton steps (rel err ~4e-6 at 2 iters).
    Safe for x == 0 (result is finite-huge, no NaN).
    """
    if eng is None:
        eng = nc.vector
    magic = pool.tile([128, 1], I32, tag=f"{tag}_magic")
    nc.vector.memset(magic, 0x5F3759DF)
    sh = pool.tile([128, n], I32, tag=f"{tag}_sh")
    eng.tensor_scalar(
        out=sh, in0=x.bitcast(I32), scalar1=1, scalar2=None,
        op0=AL.logical_shift_right,
    )
    eng.tensor_tensor(
        out=out.bitcast(I32),
        in0=magic[:, 0:1].broadcast_to((128, n)),
        in1=sh,
        op=AL.subtract,
    )
    tmp = pool.tile([128, n], F32, tag=f"{tag}_tmp")
    for _ in range(iters):
        eng.tensor_tensor(out=tmp, in0=x, in1=out, op=AL.mult)
        eng.tensor_tensor(out=tmp, in0=tmp, in1=out, op=AL.mult)
        eng.tensor_scalar(
            out=tmp, in0=tmp, scalar1=-0.5, scalar2=1.5, op0=AL.mult, op1=AL.add
        )
        eng.tensor_tensor(out=out, in0=out, in1=tmp, op=AL.mult)


def _emit_expm(nc, pool, A0, neghalf_c, out_slice, latency_mode=False):
    """A0 [128, G, 36] fp32 (g-major): skew, frob-clip, expm into out_slice
    [128, G*36] (g-major).

    Returns a list of zero-arg closures ("chunks") that emit the actual ops;
    the caller drains them interleaved with the next tile's mm1/LN emission so
    latency-critical LN ops never queue behind a full tile's expm burst.

    Internally the state lives in (x, y, g) layout (g innermost): a 6x6
    matmul product C = A @ B uses left operand L[p,i,l,g] and right operand
    R[p,j,l,g] = B^T, so the broadcast multiply is ONE op over all 4 g-groups
    with contiguous inner runs, and the l-reduction is 3 pairwise adds.
    Skew-symmetry makes the first right operand free: As^T = -As.
    """
    X16 = mybir.dt.float16   # expm state dtype: 2-byte for 2x DVE modes
    # latency_mode (final tile): keep the whole chain on the DVE so the tail
    # has no cross-engine queue hops; rsqrt switches to a Newton iteration.
    ge = nc.vector if latency_mode else nc.gpsimd

    def fl(t):
        return t[:, :, :, :].rearrange("p x y g -> p (x y g)")

    S_A = pool.tile([128, 6, 6, G], X16, tag="xS")        # (i, l, g)
    SQ = pool.tile([128, 6 * 6 * G], X16, tag="xSQ")
    Rr = pool.tile([128, 6 * G], F32, tag="xR")
    ss = pool.tile([128, G], F32, tag="xss")
    rsq = pool.tile([128, G], F32, tag="xrsq")
    scs = pool.tile([128, G], F32, tag="xscs")
    nscs = pool.tile([128, G], F32, tag="xnscs")
    As_A = pool.tile([128, 6, 6, G], X16, tag="xAsA")     # As, (i, l, g)
    As_T = pool.tile([128, 6, 6, G], X16, tag="xAsT")     # As^T = -As

    a0v = A0[:, :, :].rearrange("p g (i j) -> p g i j", i=6)

    def chunk_head():
        # skew into (i, l, g): S[p,i,l,g] = A0[p,g,i,l] - A0[p,g,l,i]
        ge.tensor_tensor(
            out=S_A[:, :, :, :],
            in0=a0v.transpose([0, 2, 3, 1]),
            in1=a0v.transpose([0, 3, 2, 1]),
            op=AL.subtract,
        )
        ge.tensor_tensor(
            out=SQ[:, :], in0=fl(S_A), in1=fl(S_A), op=AL.mult
        )
        # ss[g] = sum_{i,l} S^2: two gather reduces (DVE owns free-dim reduce)
        nc.vector.tensor_reduce(
            out=Rr[:, :],
            in_=SQ[:, :].rearrange("p (i lg) -> p lg i", i=6),
            axis=AX.X, op=AL.add,
        )
        nc.vector.tensor_reduce(
            out=ss[:, :],
            in_=Rr[:, :].rearrange("p (l g) -> p g l", l=6),
            axis=AX.X, op=AL.add,
        )
        if latency_mode:
            _emit_rsqrt(nc, pool, rsq[:, :], ss[:, :], G, "lmr", eng=nc.vector)
        else:
            nc.gpsimd.tensor_tensor(
                out=rsq[:, :], in0=ss[:, :],
                in1=neghalf_c[:, 0:1].broadcast_to((128, G)), op=AL.pow,
            )
        # scs = min(MAX_NORM * rsqrt(ss), 1) / 2^EXP_S
        nc.vector.tensor_scalar(
            out=scs[:, :], in0=rsq[:, :],
            scalar1=MAX_NORM / (1 << EXP_S), scalar2=1.0 / (1 << EXP_S),
            op0=AL.mult, op1=AL.min,
        )
        nc.vector.tensor_scalar(
            out=nscs[:, :], in0=scs[:, :], scalar1=-1.0, scalar2=None,
            op0=AL.mult,
        )
        sbc = scs[:, :].unsqueeze(1).broadcast_to((128, 36, G))
        nbc = nscs[:, :].unsqueeze(1).broadcast_to((128, 36, G))
        sv = S_A[:, :, :, :].rearrange("p i l g -> p (i l) g")
        ge.tensor_tensor(
            out=As_A[:, :, :, :].rearrange("p i l g -> p (i l) g"),
            in0=sv, in1=sbc, op=AL.mult,
        )
        ge.tensor_tensor(
            out=As_T[:, :, :, :].rearrange("p i l g -> p (i l) g"),
            in0=sv, in1=nbc, op=AL.mult,
        )

    def diag_add_one(t):
        dv = t[:, :, :, :].rearrange("p x y g -> p (x y) g")[:, 0 : 36 : 7, :]
        nc.vector.tensor_scalar(
            out=dv, in0=dv, scalar1=1.0, scalar2=None, op0=AL.add
        )

    TMP = pool.tile([128, 6, 6, 6, G], X16, tag="xTMP", bufs=2)  # (i,j,l,g)
    T2 = pool.tile([128, 36, 3 * G], X16, tag="xT2", bufs=2)     # (ij, l<3, g)

    # latency_mode: split each bprod by output row i into two parallel
    # engine chains (DVE i0:4, GPSIMD i4:6) to shrink the tail's latency.
    ISL = [(0, 4, "v"), (4, 6, "g")] if latency_mode else [(0, 6, None)]

    def bprod(out_ap, left, right, out_ap2=None):
        """out = left @ right per (token, g); out_ap (and the optional
        second output, e.g. the transposed view for the next squaring's
        right operand) is a 4-dim (p,i,j,g) view with any strides."""
        T3 = pool.tile([128, 36, G], X16, tag="xT3")
        t2v = T2[:, :, :].rearrange("p (i j) (l g) -> p i j l g", i=6, l=3)
        for i0, i1, e in ISL:
            ni = i1 - i0
            ve = nc.gpsimd if e == "g" else nc.vector
            ae = ve if e is not None else ge
            lv = (
                left[:, i0:i1, :, :].rearrange("p i l g -> p i (l g)")
                .unsqueeze(2).broadcast_to((128, ni, 6, 6 * G))
            )
            rv = (
                right[:, :, :, :].rearrange("p j l g -> p j (l g)")
                .unsqueeze(1).broadcast_to((128, ni, 6, 6 * G))
            )
            tv = TMP[:, i0:i1, :, :, :].rearrange("p i j l g -> p i j (l g)")
            ve.tensor_tensor(out=tv, in0=lv, in1=rv, op=AL.mult)
            tq = TMP[:, i0:i1, :, :, :].rearrange("p i j l g -> p (i j) l g")
            t2s = t2v[:, i0:i1, :, :, :].rearrange("p i j l g -> p (i j) l g")
            ae.tensor_tensor(
                out=t2s, in0=tq[:, :, 0:3, :], in1=tq[:, :, 3:6, :], op=AL.add
            )
            t3s = T3[:, i0 * 6 : i1 * 6, :]
            ve.tensor_tensor(
                out=t3s, in0=t2s[:, :, 0, :], in1=t2s[:, :, 1, :], op=AL.add
            )
            t2last = t2s[:, :, 2, :].rearrange("p (i j) g -> p i j g", i=ni)
            t3v_ = t3s.rearrange("p (i j) g -> p i j g", i=ni)
            ve.tensor_tensor(
                out=out_ap[:, i0:i1, :, :], in0=t3v_, in1=t2last, op=AL.add
            )
            if out_ap2 is not None:
                ve.tensor_tensor(
                    out=out_ap2[:, i0:i1, :, :], in0=t3v_, in1=t2last,
                    op=AL.add,
                )

    def rm(t):   # row-major (i, j, g) view of a state tile
        return t[:, :, :, :]

    def cm(t):   # column-major view: write C^T by swapping i/j dims
        return t[:, :, :, :].transpose([0, 2, 1, 3])

    chunks = [chunk_head]

    # Horner: M = I + As/4; for k=3..2: M = I + (As@M)/k; E = I + As@M.
    # M kept transposed (right-operand layout); As_T = M0 seed via skewness.
    M0 = pool.tile([128, 6, 6, G], X16, tag="xM0")

    def chunk_m0():
        nc.vector.tensor_scalar(
            out=fl(M0), in0=fl(As_T), scalar1=1.0 / 4.0, scalar2=None,
            op0=AL.mult,
        )
        diag_add_one(M0)

    chunks.append(chunk_m0)

    prevM = [M0]
    for k in (3, 2):
        Mk = pool.tile([128, 6, 6, G], X16, tag=f"xM{k}")

        def chunk_h(Mk=Mk, Mprev=prevM[0], k=k):
            bprod(cm(Mk), As_A, Mprev)       # Mk := (As@Mprev)^T
            nc.vector.tensor_scalar(
                out=fl(Mk), in0=fl(Mk), scalar1=1.0 / k, scalar2=None,
                op0=AL.mult,
            )
            diag_add_one(Mk)

        chunks.append(chunk_h)
        prevM[0] = Mk

    E_rm = pool.tile([128, 6, 6, G], X16, tag="xE0r")
    E_cm = pool.tile([128, 6, 6, G], X16, tag="xE0c")

    def chunk_e0(Mprev=prevM[0]):
        bprod(rm(E_rm), As_A, Mprev, out_ap2=cm(E_cm))
        diag_add_one(E_rm)
        diag_add_one(E_cm)

    chunks.append(chunk_e0)

    prevE = [(E_rm, E_cm)]
    for s in range(EXP_S):
        if s < EXP_S - 1:
            Er = pool.tile([128, 6, 6, G], X16, tag=f"xE{s+1}r")
            Ec = pool.tile([128, 6, 6, G], X16, tag=f"xE{s+1}c")

            def chunk_sq(Er=Er, Ec=Ec, prev=prevE[0]):
                bprod(rm(Er), prev[0], prev[1], out_ap2=cm(Ec))

            chunks.append(chunk_sq)
            prevE[0] = (Er, Ec)
        else:
            out_v = out_slice.rearrange("p (g i j) -> p i j g", i=6, j=6)

            def chunk_last(prev=prevE[0], out_v=out_v):
                bprod(out_v, prev[0], prev[1])

            chunks.append(chunk_last)
    return chunks


def _build(emit_pb, emit_gb):
    nc = bacc.Bacc("TRN2", target_bir_lowering=False, debug=False)

    th = nc.dram_tensor("th", [C, NT, 128, KE * TOK], B16, kind="ExternalInput")
    pwT = nc.dram_tensor("pwT", [128, KE * D], B16, kind="ExternalInput")
    wgT = nc.dram_tensor("wgT", [C, 128, KD * D], B16, kind="ExternalInput")
    wvT = nc.dram_tensor("wvT", [C, 128, KD * D], B16, kind="ExternalInput")
    woT = nc.dram_tensor("woT", [C, 128, KD * 36], B16, kind="ExternalInput")
    pb = bg = bv = None
    if emit_pb:
        pb = nc.dram_tensor("pb", [1, D], F32R, kind="ExternalInput")
    if emit_gb:
        bg = nc.dram_tensor("bg", [C, D], F32R, kind="ExternalInput")
        bv = nc.dram_tensor("bv", [C, D], F32R, kind="ExternalInput")
    out = nc.dram_tensor("out", [128, C * NT * G * 36], F32, kind="ExternalOutput")

    with tile.TileContext(nc) as tc, ExitStack() as ctx:
        singles = ctx.enter_context(tc.tile_pool(name="singles", bufs=1))
        cam = ctx.enter_context(tc.tile_pool(name="cam", bufs=2))
        tkp = ctx.enter_context(tc.tile_pool(name="tkp", bufs=2))
        work = ctx.enter_context(tc.tile_pool(name="work", bufs=2))
        xw = ctx.enter_context(tc.tile_pool(name="xw", bufs=3))
        # PSUM budget (8 banks): transposes+a0 share 1, mm1-reg 4, gate/val 3.
        tr_ps = ctx.enter_context(tc.tile_pool(name="tr_ps", bufs=2, space="PSUM"))
        reg_psp = ctx.enter_context(tc.tile_pool(name="reg_ps", bufs=3, space="PSUM"))
        mm_ps = ctx.enter_context(tc.tile_pool(name="mm_ps", bufs=3, space="PSUM"))

        # memset cannot emit float32r directly; stage in f32 and cast-copy.
        identF = singles.tile([128, 128], F32)
        make_identity(nc, identF)
        identB = singles.tile([128, 128], B16)
        nc.vector.tensor_copy(identB, identF)
        pw_s = singles.tile([128, KE, D], B16)
        pwsrc = pwT.ap().rearrange("p (k d) -> p k d", k=KE)

        for q in range(4):
            nc.gpsimd.dma_start(
                pw_s[:, q * 4 : (q + 1) * 4, :],
                pwsrc[:, q * 4 : (q + 1) * 4, :],
            )
        onesF = singles.tile([1, TOK], F32)
        nc.vector.memset(onesF, 1.0)
        ones128 = singles.tile([1, 128], F32R)
        nc.vector.tensor_copy(ones128, onesF[:, :128])
        ones512 = singles.tile([1, TOK], F32R)
        nc.vector.tensor_copy(ones512, onesF)
        pb_s = None
        if emit_pb:
            pb_s = singles.tile([1, D], F32R)
            nc.gpsimd.dma_start(pb_s, pb.ap())
        # all 16 tiles' results accumulate here; one DMA at the end keeps the
        # SP queue free of expm-dependent waits mid-stream.
        eacc = singles.tile([128, C * NT, G * 36], F32)
        # gpsimd rsqrt via vpowf: pow(x, -0.5)
        neghalf = singles.tile([128, 1], F32)
        nc.vector.memset(neghalf, -0.5)
        negone = singles.tile([128, 1], F32)
        nc.vector.memset(negone, -1.0)
        epsB = singles.tile([128, 1], F32)
        nc.vector.memset(epsB, LN_EPS)
        invD = singles.tile([128, 1], F32)
        nc.vector.memset(invD, 1.0 / D)

        cam_state = {}

        def stage_a(step, tr_hook=None):
            """thumbnail DMA + mm1 + LayerNorm for tile `step` -> y."""
            c, ti = divmod(step, NT)
            if ti == 0:
                wg_s = cam.tile([128, KD, D], B16, tag="wg")
                nc.gpsimd.dma_start(
                    wg_s, wgT.ap()[c].rearrange("p (k f) -> p k f", k=KD)
                )
                wv_s = cam.tile([128, KD, D], B16, tag="wv")
                nc.gpsimd.dma_start(
                    wv_s, wvT.ap()[c].rearrange("p (k f) -> p k f", k=KD)
                )
                wo_s = cam.tile([128, KD, 36], B16, tag="wo")
                nc.gpsimd.dma_start(
                    wo_s, woT.ap()[c].rearrange("p (k o) -> p k o", k=KD)
                )
                bg_s = bv_s = None
                if emit_gb:
                    bg_s = cam.tile([1, D], F32R, tag="bg")
                    nc.gpsimd.dma_start(bg_s, bg.ap()[c : c + 1, :])
                    bv_s = cam.tile([1, D], F32R, tag="bv")
                    nc.gpsimd.dma_start(bv_s, bv.ap()[c : c + 1, :])
                cam_state[c] = (wg_s, wv_s, wo_s, bg_s, bv_s)

            # thumbnails arrive host-blocked [C, NT, 128, KE*TOK]: one DMA
            # with one contiguous run per partition, no on-chip transpose.
            thsrc = th.ap()[c, ti].rearrange("p (k b) -> p k b", k=KE)
            q = KE // 4
            ths = []
            for j in range(4):
                t_ = tkp.tile([128, q, TOK], B16, tag=f"tk{j}")
                nc.sync.dma_start(t_, thsrc[:, j * q : (j + 1) * q, :])
                ths.append(t_)

            # ---- mm1 (m-outer, one PSUM bank per token chunk) + LayerNorm.
            # Per chunk: 16 accumulating matmuls, bn stats on DVE, rsqrt
            # chain on GPSIMD, then y = rstd*reg - mu*rstd on ACT (frees
            # the bank early so chunks pipeline through 3 reg banks).
            y = work.tile([128, G, D], B16, tag="y")
            rstd = work.tile([128, G], F32, tag="rstd")
            for g in range(G):
                reg_ps = reg_psp.tile([128, D], F32, tag="reg")
                for k in range(KE):
                    tsrc = ths[k // q]
                    nc.tensor.matmul(
                        reg_ps,
                        tsrc[:, k % q, g * 128 : (g + 1) * 128],
                        pw_s[:, k, :],
                        start=(k == 0),
                        stop=(k == KE - 1 and not emit_pb),
                    )
                if emit_pb:
                    nc.tensor.matmul(reg_ps, ones128, pb_s, start=False, stop=True)
                st = work.tile([128, 6], F32, tag="bst")
                nc.vector.bn_stats(out=st[:, :], in_=reg_ps)
                mv = work.tile([128, 2], F32, tag="mv")
                nc.vector.bn_aggr(out=mv[:, :], in_=st[:, :])
                negmu = work.tile([128, 1], F32, tag="negmu")
                nc.vector.tensor_scalar(
                    out=negmu, in0=mv[:, 0:1], scalar1=-1.0,
                    scalar2=None, op0=AL.mult,
                )
                # rstd = pow(var+eps, -0.5) via gpsimd vpowf, then
                # y = rstd*reg - mu*rstd in one ACT op (single rounding,
                # frees this chunk's PSUM bank right away)
                vpe = work.tile([128, 1], F32, tag="vpe")
                nc.gpsimd.tensor_tensor(out=vpe, in0=mv[:, 1:2], in1=epsB, op=AL.add)
                nc.gpsimd.tensor_tensor(
                    out=rstd[:, g : g + 1], in0=vpe, in1=neghalf, op=AL.pow
                )
                nmr = work.tile([128, 1], F32, tag="nmr")
                nc.gpsimd.tensor_tensor(
                    out=nmr, in0=negmu, in1=rstd[:, g : g + 1], op=AL.mult
                )
                if g == 0 and tr_hook is not None:
                    # previous tile's transposes + yT copies: their deps are a
                    # tile old, so they fill the ACT queue ahead of y(t) and
                    # free the tr PSUM banks early
                    tr_hook()
                nc.scalar.activation(
                    y[:, g, :], reg_ps, AF.Identity,
                    bias=nmr, scale=rstd[:, g : g + 1],
                )
            return {"c": c, "ti": ti, "y": y}

        def stage_b1(st):
            """previous tile's yT transposes + copies (PE + ACT)."""
            y = st["y"]
            yTs = []
            for kd in range(KD):
                tr = tr_ps.tile([128, TOK], B16, tag="tr")
                for g in range(G):
                    nc.tensor.transpose(
                        tr[:, g * 128 : (g + 1) * 128],
                        y[:, g, kd * 128 : (kd + 1) * 128],
                        identB,
                    )
                yT = work.tile([128, TOK], B16, tag=f"yT{kd}", bufs=1)
                # kd3's copy gates the next tile's first transpose via PSUM
                # bank reuse: keep it on the prompt ACT lane, rest on DVE
                if kd == KD - 1:
                    nc.scalar.copy(yT[:, :], tr)
                else:
                    nc.vector.tensor_copy(yT[:, :], tr)
                yTs.append(yT)
            st["yTs"] = yTs

        def stage_b2(st, latency_mode=False):
            """SwiGLU + out-proj + expm for a completed tile."""
            c, ti = st["c"], st["ti"]
            yTs = st["yTs"]
            wg_s, wv_s, wo_s, bg_s, bv_s = cam_state[c]

            # ---- mm2 gate/val + silu + h
            hs = []
            for mf in range(KD):
                g_ps = mm_ps.tile([128, TOK], F32, tag="mm")
                for kd in range(KD):
                    nc.tensor.matmul(
                        g_ps,
                        wg_s[:, kd, mf * 128 : (mf + 1) * 128],
                        yTs[kd][:, :],
                        start=(kd == 0),
                        stop=(kd == KD - 1 and not emit_gb),
                    )
                if emit_gb:
                    nc.tensor.matmul(
                        g_ps, bg_s[:, mf * 128 : (mf + 1) * 128], ones512,
                        start=False, stop=True,
                    )
                v_ps = mm_ps.tile([128, TOK], F32, tag="mm")
                for kd in range(KD):
                    nc.tensor.matmul(
                        v_ps,
                        wv_s[:, kd, mf * 128 : (mf + 1) * 128],
                        yTs[kd][:, :],
                        start=(kd == 0),
                        stop=(kd == KD - 1 and not emit_gb),
                    )
                if emit_gb:
                    nc.tensor.matmul(
                        v_ps, bv_s[:, mf * 128 : (mf + 1) * 128], ones512,
                        start=False, stop=True,
                    )
                sg = work.tile([128, TOK], F32, tag="sg", bufs=2)
                nc.scalar.activation(sg, g_ps, AF.Silu)
                h = work.tile([128, TOK], B16, tag=f"h{mf}", bufs=1)
                nc.vector.tensor_tensor(out=h[:, :], in0=sg, in1=v_ps, op=AL.mult)
                hs.append(h)

            # ---- mm3: A0T[36, t] = woT.T @ h (4 wide fp32r matmuls),
            # then PE-transpose back to [t, 36] for the expm stage.
            a0_ps = tr_ps.tile([128, G, 36], B16, tag="tr")
            a0T_ps = mm_ps.tile([36, TOK], F32, tag="mm", name="a0T")
            for kf in range(KD):
                nc.tensor.matmul(
                    a0T_ps,
                    wo_s[:, kf, :],
                    hs[kf][:, :],
                    start=(kf == 0),
                    stop=(kf == KD - 1),
                )
            a0T_s = work.tile([36, TOK], B16, tag="a0T")
            nc.scalar.copy(a0T_s, a0T_ps)
            for g in range(G):
                nc.tensor.transpose(
                    a0_ps[:, g, :],
                    a0T_s[:, g * 128 : (g + 1) * 128],
                    identB[:36, :36],
                )
            A0 = xw.tile([128, G, 36], mybir.dt.float16, tag="A0")
            if latency_mode:
                nc.vector.tensor_copy(A0[:, :, :], a0_ps[:, :, :])
            else:
                nc.scalar.copy(A0[:, :, :], a0_ps[:, :, :])
            return _emit_expm(
                nc, xw, A0, neghalf, out_slice=eacc[:, c * NT + ti, :],
                latency_mode=latency_mode,
            )

        # 1-stage software pipeline: tile t's post-mm1 work (PE transposes,
        # mm2/mm3) is emitted after tile t+1's mm1+LN, so the PE never waits
        # on the LN chain.  The expm of tile t is emitted as chunks
        # interleaved into tile t+2's mm1/LN g-loop so latency-critical LN
        # ops never queue behind a full expm burst on any in-order engine.
        pending = []          # (tile_idx, closure) FIFO
        cur_step = [0]

        def drain(k, force=False):
            for _ in range(k):
                if not pending:
                    return
                if not force and pending[0][0] > cur_step[0] - 2:
                    return
                pending.pop(0)[1]()

        prev = None
        for step in range(C * NT):
            cur_step[0] = step
            if step >= 6 and (step - 6) % NT == 0:
                cc = (step - 6) // NT
                nc.sync.dma_start(
                    out.ap()[:, cc * NT * G * 36 : (cc + 1) * NT * G * 36],
                    eacc[:, cc * NT : (cc + 1) * NT, :].rearrange(
                        "p t a -> p (t a)"
                    ),
                )
            hook = (lambda p=prev: stage_b1(p)) if prev is not None else None
            cur = stage_a(step, tr_hook=hook)
            if prev is not None:
                pending.extend(
                    (step - 1, ch) for ch in stage_b2(prev)
                )
            drain(len(pending))
            prev = cur
        drain(len(pending), force=True)   # tile 14's expm overlaps tile 15's PE
        stage_b1(prev)
        pending.extend(
            (C * NT - 1, ch) for ch in stage_b2(prev, latency_mode=True)
        )
        drain(len(pending), force=True)

        nc.sync.dma_start(
            out.ap()[:, 3 * NT * G * 36 :],
            eacc[:, 3 * NT :, :].rearrange("p t a -> p (t a)"),
        )

    nc.compile()
    return nc


def kernel(**inputs):
    global last_results, last_in_maps
    thumb = np.asarray(inputs["thumbnails"], dtype=np.float32)
    # [B, C, E] -> per-core [C, NT, 128, KE, TOK] so each tile lands with one
    # DMA whose per-partition data is contiguous (128 descriptors, not 2048).
    thB = thumb.reshape(N_CORES, NT, TOK, C, KE, 128)
    thB = np.ascontiguousarray(thB.transpose(0, 3, 1, 5, 4, 2))
    proj_w = np.asarray(inputs["proj_w"], dtype=np.float32)
    proj_b = np.asarray(inputs["proj_b"], dtype=np.float32)
    gamma = np.asarray(inputs["gamma"], dtype=np.float32)
    beta = np.asarray(inputs["beta"], dtype=np.float32)
    w_gate = np.asarray(inputs["w_gate"], dtype=np.float32)
    w_val = np.asarray(inputs["w_val"], dtype=np.float32)
    w_out = np.asarray(inputs["w_out"], dtype=np.float32)

    # host-side weight prep: fold gamma into the SwiGLU weights, beta into
    # rank-1 biases, pre-transpose everything for the PE's lhsT convention.
    def blockT(w):
        # [out, in] -> in-major [128, k, out]: lhsT chunks contiguous/partition
        o, i = w.shape
        return np.ascontiguousarray(
            w.T.reshape(i // 128, 128, o).transpose(1, 0, 2)
        ).reshape(128, i // 128 * o)

    BF = ml_dtypes.bfloat16
    pwT = blockT(proj_w).astype(BF)                             # [128, KE*D]
    wgT = np.stack(
        [blockT(w_gate[c] * gamma[c][None, :]) for c in range(C)]
    ).astype(BF)
    wvT = np.stack(
        [blockT(w_val[c] * gamma[c][None, :]) for c in range(C)]
    ).astype(BF)
    woT = np.stack([blockT(w_out[c]) for c in range(C)]).astype(BF)
    bg = np.einsum("cfd,cd->cf", w_gate, beta).astype(np.float32)
    bv = np.einsum("cfd,cd->cf", w_val, beta).astype(np.float32)

    emit_pb = bool(np.any(proj_b))
    emit_gb = bool(np.any(bg) or np.any(bv))

    key = (emit_pb, emit_gb)
    if key not in _BUILD_CACHE:
        _BUILD_CACHE[key] = _build(emit_pb, emit_gb)
    nc = _BUILD_CACHE[key]

    shared = {"pwT": pwT, "wgT": wgT, "wvT": wvT, "woT": woT}
    if emit_pb:
        shared["pb"] = proj_b.reshape(1, D)
    if emit_gb:
        shared["bg"] = bg
        shared["bv"] = bv
    in_maps = []
    for i in range(N_CORES):
        m = dict(shared)
        m["th"] = thB[i].reshape(C, NT, 128, KE * TOK).astype(ml_dtypes.bfloat16)
        in_maps.append(m)

    last_in_maps = in_maps
    trace = bool(int(os.environ.get("KERNEL_TRACE", "0")))
    try:
        last_results = run_bass_kernel_spmd(
            nc, in_maps, core_ids=list(range(N_CORES)), trace=trace
        )
    except ModuleNotFoundError:
        # tracing requested (e.g. BASS_TRACE in env) but the axon NTFF hook
        # module is absent in this image -- rerun without tracing.
        os.environ["BASS_NEVER_TRACE"] = "1"
        last_results = run_bass_kernel_spmd(
            nc, in_maps, core_ids=list(range(N_CORES)), trace=False
        )
    parts = []
    for r in last_results.results:
        o = r["out"].reshape(128, C, NT, G, 36)
        parts.append(o.transpose(1, 2, 3, 0, 4).reshape(C, BLOC, 36))
    full = np.concatenate(parts, axis=1)                        # [C, B, 36]
    return full.reshape(C, B, 6, 6)

